# revision 1
# baseline (speedup 1.0000x reference)
"""LightGCN-style GNN (3 mean-agg layers + review conv + edge-softmax attention)
on 8 Trainium2 NeuronCores.

Strategy: shard every phase by destination rows (8 contiguous ranges).  Each
core gathers source rows with int16-chunked `dma_gather`, reduces segments with
one-hot matmuls accumulated in PSUM (128-dst subwindows), normalizes with
host-precomputed inverse counts, and writes its shard.  Full tables needed by
the next phase are rebuilt with AllGather collectives.  All index manipulation
(sorting edges into (superwindow, chunk, subwindow) segments, int16 packing,
degree counts) happens on the host; all FLOPs and feature movement happen on
device.
"""

import os
import sys
import types

import numpy as np

# ---------------------------------------------------------------------------
# configuration (overridable for scaled-down testing)
# ---------------------------------------------------------------------------
CFG = {
    "R": 400_000,      # review nodes
    "M": 100_000,      # final dst nodes
    "L": 3,            # propagation layers
    "NCORE": 8,
    "CH": 32768,       # int16 gather chunk
    "SUB": 128,        # dst rows per subwindow (PSUM partitions)
    "NSUP": 16,        # subwindows per superwindow (e1/e2)
    "NSUP3": 8,        # subwindows per superwindow (e3; wider PSUM slots)
    "OHG": 8,          # one-hot build group (blocks per DVE op)
    "NQ": 4,           # SWDGE queues
    "TRACE": False,
}

_LAST = {"exec_ns": None, "profile_json": None}


def _install_profile_hook():
    try:
        if "antenv.axon_hooks" in sys.modules:
            return
        import antenv

        mod = types.ModuleType("antenv.axon_hooks")
        mod._hook = None
        mod.set_axon_ntff_profile_hook = lambda h: setattr(mod, "_hook", h)
        mod.get_axon_ntff_profile_hook = lambda: mod._hook
        sys.modules["antenv.axon_hooks"] = mod
        antenv.axon_hooks = mod
        from trn_agent_boot.trn_boot import _ntff_profile_via_ctypes

        mod.set_axon_ntff_profile_hook(
            _ntff_profile_via_ctypes("/opt/axon/libaxon_pjrt.so")
        )
    except Exception:
        pass


# ---------------------------------------------------------------------------
# host-side index preparation
# ---------------------------------------------------------------------------
class PhaseMeta:
    """Static (core-independent) structure of one gather/reduce phase."""

    def __init__(self, nsub, nsup, nchunk, table_rows, caps):
        self.nsub = nsub            # total subwindows (padded to nsup multiple)
        self.nsup = nsup
        self.nchunk = nchunk
        self.table_rows = table_rows
        self.caps = caps            # [nsub, nchunk] slot capacity (mult of 128)
        self.nsuper = nsub // nsup
        # piece (s, c) capacities & segment offsets
        self.seg_off = np.zeros((nsub, nchunk), np.int64)  # piece-local slot off
        self.piece_cap = np.zeros((self.nsuper, nchunk), np.int64)
        for s in range(self.nsuper):
            w0 = s * nsup
            for c in range(nchunk):
                off = 0
                for wl in range(nsup):
                    self.seg_off[w0 + wl, c] = off
                    off += caps[w0 + wl, c]
                self.piece_cap[s, c] = off
        # global slot base of each piece, pieces ordered (s, c)
        self.piece_base = np.zeros((self.nsuper, nchunk), np.int64)
        b = 0
        for s in range(self.nsuper):
            for c in range(nchunk):
                self.piece_base[s, c] = b
                b += self.piece_cap[s, c]
        self.total_slots = b
        self.w_has_edges = caps.sum(1) > 0

    def edge_slots(self, dstloc, srcflat):
        """Map per-core edges to absolute slots; returns (slot, idx16val, dloc)."""
        w = dstloc >> 7
        c = srcflat // CFG["CH"]
        s = w // self.nsup
        # piece-major, then subwindow, stable order
        key = (s * self.nchunk + c) * self.nsub + w
        order = np.argsort(key, kind="stable")
        ks = key[order]
        # rank within equal keys
        change = np.empty(len(ks), bool)
        if len(ks):
            change[0] = True
            change[1:] = ks[1:] != ks[:-1]
        starts = np.flatnonzero(change)
        rank = np.arange(len(ks)) - np.repeat(starts, np.diff(np.append(starts, len(ks))))
        wo, co, so = w[order], c[order], s[order]
        slot = self.piece_base[so, co] + self.seg_off[wo, co] + rank
        return order, slot


def _phase_structure(percore_edges, nsub, nsup, nchunk):
    """percore_edges: list of (dstloc, srcflat) -> caps [nsub, nchunk]."""
    ncore = len(percore_edges)
    cnts = np.zeros((ncore, nsub * nchunk), np.int64)
    for i, (dl, sf) in enumerate(percore_edges):
        seg = (dl >> 7) * nchunk + sf // CFG["CH"]
        cnts[i] = np.bincount(seg, minlength=nsub * nchunk)
    caps = cnts.max(0)
    caps = ((caps + 127) // 128) * 128
    return caps.reshape(nsub, nchunk)


def _pack_core_data(meta, dstloc, srcflat):
    """Returns idx16 [128, total/16] int16, dloc [128, total/128] f32."""
    T = meta.total_slots
    idxval = np.zeros(T, np.int16)
    dval = np.full(T, -1.0, np.float32)
    if len(dstloc):
        order, slot = meta.edge_slots(dstloc, srcflat)
        idxval[slot] = (srcflat[order] % CFG["CH"]).astype(np.int16)
        dval[slot] = (dstloc[order] & 127).astype(np.float32)
    # pack idx16: per 128-slot col a, slot e=a*128+j*16+cc -> [16cc, a*8+j]
    A = T // 128
    m = idxval.reshape(A * 8, 16).T                  # [16, A*8]
    idx16 = np.tile(m, (8, 1))                       # [128, A*8]
    dloc = dval.reshape(A, 128).T.copy()             # [128, A]
    return idx16, dloc


def _invcnt_pmajor(dstloc, nsub):
    cnt = np.bincount(dstloc, minlength=nsub * 128)
    inv = 1.0 / np.maximum(cnt, 1)
    return inv.reshape(nsub, 128).T.astype(np.float32).copy()


def _pmajor_rowmap(nsub):
    """global-local row r -> flat row p*nsub + w  of a [128, nsub, D] table."""

    def f(r):
        return (r % 128) * nsub + (r // 128)

    return f


# ---------------------------------------------------------------------------
# device kernel builder
# ---------------------------------------------------------------------------
def _emit_phase(nc, tile, pools, meta, src_view, idx_t, dloc_t, out_tile,
                invcnt_t=None, iota_t=None, e3=None, qstate=None, D=64):
    """Emit one gather/one-hot-reduce phase.  e3 = (vrep_tile, crep_tile)."""
    import concourse.mybir as mybir

    f32 = mybir.dt.float32
    CH = CFG["CH"]
    nsup = meta.nsup
    slotw = D if e3 is None else 2 * D
    slots_per_bank = 512 // slotw
    nbanks = (nsup + slots_per_bank - 1) // slots_per_bank
    OHG = CFG["OHG"]

    for s in range(meta.nsuper):
        banks = [pools["psum"].tile([128, 512], f32, tag="bank", name=f"bank{bi}")
                 for bi in range(nbanks)]
        for bk in banks:
            nc.vector.memset(bk[:], 0.0)

        def bank_slice(wl, lo, hi):
            b = wl // slots_per_bank
            off = (wl % slots_per_bank) * slotw
            return banks[b][:, off + lo:off + hi]

        # how many blocks feed each subwindow of this super (for start/stop)
        blk_total = {wl: int(meta.caps[s * nsup + wl, :].sum() // 128)
                     for wl in range(nsup)}
        blk_seen = {wl: 0 for wl in range(nsup)}

        for c in range(meta.nchunk):
            cap = int(meta.piece_cap[s, c])
            if cap == 0:
                continue
            A = cap // 128
            base = int(meta.piece_base[s, c])
            it = pools["idx"].tile([128, cap // 16], mybir.dt.int16, tag="idx")
            nc.sync.dma_start(out=it[:], in_=idx_t[:, base // 16:base // 16 + cap // 16])
            dl = pools["dloc"].tile([128, A], f32, tag="dloc")
            nc.sync.dma_start(out=dl[:], in_=dloc_t[:, base // 128:base // 128 + A])
            gt = pools["gather"].tile([128, A, D], f32, tag="gt")
            lo, hi = c * CH, min((c + 1) * CH, meta.table_rows)
            nc.gpsimd.dma_gather(
                out_ap=gt[:], in_ap=src_view[lo:hi], idxs_ap=it[:],
                num_idxs=cap, num_idxs_reg=cap, elem_size=D,
                queue_num=qstate[0] % CFG["NQ"], single_packet=False,
            )
            qstate[0] += 1

            if e3 is not None:
                vrep, crep = e3
                tmp = pools["tmp"].tile([128, A, D], f32, tag="tmp")
                nc.vector.tensor_tensor(
                    out=tmp[:], in0=gt[:],
                    in1=vrep[:].rearrange("p (o d) -> p o d", o=1).to_broadcast([128, A, D]),
                    op=mybir.AluOpType.mult)
                ze = pools["ze"].tile([128, A], f32, tag="ze")
                nc.vector.tensor_reduce(out=ze[:], in_=tmp[:],
                                        axis=mybir.AxisListType.X,
                                        op=mybir.AluOpType.add)
                nc.scalar.activation(out=ze[:], in_=ze[:],
                                     func=mybir.ActivationFunctionType.Exp,
                                     bias=crep[:, 0:1], scale=1.0)
                tmpb = pools["tmpb"].tile([128, A, D], mybir.dt.bfloat16,
                                          tag="tmpb")
                nc.vector.tensor_tensor(
                    out=tmpb[:], in0=gt[:],
                    in1=ze[:].rearrange("p (a o) -> p a o", o=1).to_broadcast([128, A, D]),
                    op=mybir.AluOpType.mult)
                zeb = pools["zeb"].tile([128, A], mybir.dt.bfloat16, tag="zeb")
                nc.vector.tensor_copy(out=zeb[:], in_=ze[:])
                rhs_feats, rhs_den = tmpb, zeb
            else:
                gtb = pools["tmpb"].tile([128, A, D], mybir.dt.bfloat16,
                                         tag="tmpb")
                nc.vector.tensor_copy(out=gtb[:], in_=gt[:])
                rhs_feats, rhs_den = gtb, None

            # one-hot groups + matmuls
            blocks = []  # (a, w_local)
            for wl in range(nsup):
                w = s * nsup + wl
                nb = int(meta.caps[w, c] // 128)
                off = int(meta.seg_off[w, c])
                for b in range(nb):
                    blocks.append(((off + b * 128) // 128, wl))
            blocks.sort()
            gi = 0
            while gi < len(blocks):
                g = blocks[gi:gi + OHG]
                a0 = g[0][0]
                ga = g[-1][0] - a0 + 1
                oh = pools["oh"].tile([128, OHG, 128], mybir.dt.bfloat16, tag="oh")
                nc.vector.tensor_tensor(
                    out=oh[:, :ga, :],
                    in0=iota_t[:].rearrange("p (o x) -> p o x", o=1).to_broadcast([128, ga, 128]),
                    in1=dl[:, a0:a0 + ga].rearrange("p (a o) -> p a o", o=1).to_broadcast([128, ga, 128]),
                    op=mybir.AluOpType.is_equal)
                for a, wl in g:
                    blk_seen[wl] += 1
                    last = blk_seen[wl] == blk_total[wl]
                    nc.tensor.matmul(
                        out=bank_slice(wl, 0, D), lhsT=oh[:, a - a0, :],
                        rhs=rhs_feats[:, a, :], start=False, stop=last,
                        skip_group_check=True)
                    if rhs_den is not None:
                        nc.tensor.matmul(
                            out=bank_slice(wl, D, D + 1), lhsT=oh[:, a - a0, :],
                            rhs=rhs_den[:, a:a + 1], start=False, stop=last,
                            skip_group_check=True)
                gi += len(g)

        # normalize + stage out
        stage = pools["stage"].tile([128, nsup * D], f32, tag="stage")
        if invcnt_t is not None:
            ic = pools["ic"].tile([128, nsup], f32, tag="ic")
            nc.sync.dma_start(out=ic[:], in_=invcnt_t[:, s * nsup:(s + 1) * nsup])
        for wl in range(nsup):
            w = s * nsup + wl
            dst = stage[:, wl * D:(wl + 1) * D]
            if not meta.w_has_edges[w]:
                nc.vector.memset(dst, 0.0)
                continue
            if e3 is None:
                nc.vector.tensor_scalar(
                    out=dst, in0=bank_slice(wl, 0, D),
                    scalar1=ic[:, wl:wl + 1], scalar2=None,
                    op0=mybir.AluOpType.mult)
            else:
                dt = pools["den"].tile([128, 1], f32, tag="den")
                nc.vector.tensor_scalar(
                    out=dt[:], in0=bank_slice(wl, D, D + 1),
                    scalar1=1e-9, scalar2=None, op0=mybir.AluOpType.max)
                nc.vector.reciprocal(out=dt[:], in_=dt[:])
                nc.vector.tensor_scalar(
                    out=dst, in0=bank_slice(wl, 0, D),
                    scalar1=dt[:, 0:1], scalar2=None,
                    op0=mybir.AluOpType.mult)
        nc.sync.dma_start(
            out=out_tile[:, s * nsup:(s + 1) * nsup, :],
            in_=stage[:].rearrange("p (w d) -> p w d", d=D))


def kernel(**inputs):
    _install_profile_hook()
    import concourse.bacc as bacc
    import concourse.mybir as mybir
    import concourse.tile as tile
    from concourse.bass_utils import run_bass_kernel_spmd

    f32 = mybir.dt.float32

    emb = np.asarray(inputs["emb_table"], np.float32)
    node_ids = np.asarray(inputs["node_ids"])
    w_o = np.asarray(inputs["w_o"], np.float32)
    b_o = np.asarray(inputs["b_o"], np.float32)
    att_w = np.asarray(inputs["att_w"], np.float32)
    att_b = np.asarray(inputs["att_b"], np.float32)
    e1_src = np.asarray(inputs["e1_src"], np.int64)
    e1_dst = np.asarray(inputs["e1_dst"], np.int64)
    e2_src = np.asarray(inputs["e2_src"], np.int64)
    e2_dst = np.asarray(inputs["e2_dst"], np.int64)
    e3_src = np.asarray(inputs["e3_src"], np.int64)
    e3_dst = np.asarray(inputs["e3_dst"], np.int64)

    N, D = emb.shape
    R, M, L = CFG["R"], CFG["M"], CFG["L"]
    NC, CH, SUB, NSUP, NSUP3 = (CFG["NCORE"], CFG["CH"], CFG["SUB"],
                                CFG["NSUP"], CFG["NSUP3"])

    x0 = emb[node_ids]                      # [N, D] (node_ids is arange per spec)
    v = (w_o @ att_w).astype(np.float32).ravel()          # [D]
    c_sc = float(b_o @ att_w.ravel() + att_b.ravel()[0])  # scalar

    NSH = N // NC
    MSH = M // NC
    nsub1 = -(-NSH // 128)
    nsub1 = -(-nsub1 // NSUP) * NSUP          # padded subwindows per core
    rows_x = NC * 128 * nsub1                 # p-major full-table rows
    map_x = _pmajor_rowmap(nsub1)

    nsub3 = -(-MSH // 128)
    nsub3 = -(-nsub3 // NSUP3) * NSUP3

    # ---------------- e1 edges per core (dst-range shard) -----------------
    core_of1 = np.minimum(e1_dst // NSH, NC - 1)
    e1_by_core = []
    for i in range(NC):
        m = core_of1 == i
        e1_by_core.append((e1_dst[m] - i * NSH, e1_src[m]))
    caps1_l1 = _phase_structure([(d, s) for d, s in e1_by_core], nsub1, NSUP,
                                -(-N // CH))
    meta1_l1 = PhaseMeta(nsub1, NSUP, -(-N // CH), N, caps1_l1)
    # layers 2..L gather from p-major x tables
    e1_by_core_pm = []
    for d, s in e1_by_core:
        ci = np.minimum(s // NSH, NC - 1)
        r = s - ci * NSH
        flat = (ci * 128 + (r % 128)) * nsub1 + r // 128
        e1_by_core_pm.append((d, flat))
    caps1_lx = _phase_structure(e1_by_core_pm, nsub1, NSUP, -(-rows_x // CH))
    meta1_lx = PhaseMeta(nsub1, NSUP, -(-rows_x // CH), rows_x, caps1_lx)

    # ---------------- e2: consumer-sharded reviews ------------------------
    e2cnt = np.bincount(e2_dst, minlength=R)          # global review in-degree
    core_of3 = np.minimum(e3_dst // MSH, NC - 1)
    # chunk signature ordering for packing (based on xbar p-major rows)
    ci2 = np.minimum(e2_src // NSH, NC - 1)
    r2 = e2_src - ci2 * NSH
    e2_srcflat = (ci2 * 128 + (r2 % 128)) * nsub1 + r2 // 128
    e2_chunk = e2_srcflat // CH

    cons_lists, e2_data, e3_data, inv2_list = [], [], [], []
    # per-review (min, max) source chunk — sort key for pack-friendly numbering
    o2 = np.lexsort((e2_chunk, e2_dst))
    e2d_s, e2c_s = e2_dst[o2], e2_chunk[o2]
    rstart = np.searchsorted(e2d_s, np.arange(R + 1))
    cmin = np.full(R, 99, np.int64)
    cmax = np.full(R, 99, np.int64)
    has = rstart[1:] > rstart[:-1]
    if len(e2c_s):
        cmin[has] = e2c_s[rstart[:-1][has]]
        cmax[has] = e2c_s[rstart[1:][has] - 1]

    for i in range(NC):
        m3 = core_of3 == i
        src3 = e3_src[m3]
        dst3 = e3_dst[m3] - i * MSH
        cons = np.unique(src3)
        # pack-friendly ordering: by (cmin, cmax) of each review's e2 edges
        key = cmin[cons].astype(np.int64) * 100 + cmax[cons]
        cons = cons[np.argsort(key, kind="stable")]
        lid = np.full(R, -1, np.int64)
        lid[cons] = np.arange(len(cons))
        cons_lists.append(cons)
        sel = lid[e2_dst] >= 0
        e2_data.append((lid[e2_dst[sel]], e2_srcflat[sel]))
        e3_data.append((dst3, lid[src3]))
        inv2 = 1.0 / np.maximum(e2cnt[cons], 1)
        inv2_list.append(inv2.astype(np.float32))

    revcap = max(len(c) for c in cons_lists)
    nsub2 = -(-revcap // 128)
    nsub2 = -(-nsub2 // NSUP) * NSUP
    rows_rev = 128 * nsub2
    map_rev = _pmajor_rowmap(nsub2)

    caps2 = _phase_structure(e2_data, nsub2, NSUP, -(-rows_x // CH))
    meta2 = PhaseMeta(nsub2, NSUP, -(-rows_x // CH), rows_x, caps2)

    e3_data_pm = [(d, map_rev(s)) for d, s in e3_data]
    caps3 = _phase_structure(e3_data_pm, nsub3, NSUP3, -(-rows_rev // CH))
    meta3 = PhaseMeta(nsub3, NSUP3, -(-rows_rev // CH), rows_rev, caps3)

    # ---------------- per-core input arrays -------------------------------
    in_maps = []
    for i in range(NC):
        d1, s1 = e1_by_core[i]
        idxA, dlA = _pack_core_data(meta1_l1, d1, s1)
        d1x, s1x = e1_by_core_pm[i]
        idxB, dlB = _pack_core_data(meta1_lx, d1x, s1x)
        inv1 = _invcnt_pmajor(d1, nsub1)
        d2, s2 = e2_data[i]
        idx2, dl2 = _pack_core_data(meta2, d2, s2)
        inv2 = np.zeros((128, nsub2), np.float32)
        li = np.arange(len(cons_lists[i]))
        inv2[li % 128, li // 128] = inv2_list[i]
        d3, s3 = e3_data_pm[i]
        idx3, dl3 = _pack_core_data(meta3, d3, s3)
        # emb_local p-major [128, nsub1, D]
        embl = np.zeros((128, nsub1, D), np.float32)
        loc = x0[i * NSH:(i + 1) * NSH]
        r = np.arange(NSH)
        embl[r % 128, r // 128] = loc
        in_maps.append({
            "emb": np.ascontiguousarray(x0),
            "emb_local": embl,
            "idx_l1": idxA, "dl_l1": dlA,
            "idx_lx": idxB, "dl_lx": dlB,
            "inv1": inv1,
            "idx_e2": idx2, "dl_e2": dl2, "inv2": inv2,
            "idx_e3": idx3, "dl_e3": dl3,
            "iota": np.tile(np.arange(128, dtype=np.float32), (128, 1)),
            "vrep": np.tile(v, (128, 1)).astype(np.float32),
            "crep": np.full((128, 1), c_sc, np.float32),
        })

    # ---------------- build device program --------------------------------
    nc = bacc.Bacc("TRN2", target_bir_lowering=False, debug=False,
                   num_devices=NC, num_swdge_queues=CFG["NQ"])

    def din(name, arr):
        return nc.dram_tensor(name, list(arr.shape),
                              mybir.dt.from_np(arr.dtype), kind="ExternalInput")

    t = {k: din(k, in_maps[0][k]) for k in in_maps[0]}
    out_t = nc.dram_tensor("out", [128, nsub3, D], f32, kind="ExternalOutput")
    dbg_t = {}
    if CFG.get("DEBUG"):
        for nm, sh in (("d_x1", [128, nsub1, D]), ("d_x2", [128, nsub1, D]),
                       ("d_x3", [128, nsub1, D]), ("d_xbar", [128, nsub1, D]),
                       ("d_rev", [128, nsub2, D])):
            dbg_t[nm] = nc.dram_tensor(nm, sh, f32, kind="ExternalOutput")

    qstate = [0]
    with tile.TileContext(nc) as tc:
        with (
            tc.tile_pool(name="psum", bufs=6, space="PSUM") as psum_p,
            tc.tile_pool(name="gather", bufs=5) as gather_p,
            tc.tile_pool(name="idx", bufs=5) as idx_p,
            tc.tile_pool(name="dloc", bufs=5) as dloc_p,
            tc.tile_pool(name="oh", bufs=4) as oh_p,
            tc.tile_pool(name="stage", bufs=3) as stage_p,
            tc.tile_pool(name="ic", bufs=3) as ic_p,
            tc.tile_pool(name="tmp", bufs=3) as tmp_p,
            tc.tile_pool(name="tmpb", bufs=4) as tmpb_p,
            tc.tile_pool(name="zeb", bufs=3) as zeb_p,
            tc.tile_pool(name="ze", bufs=3) as ze_p,
            tc.tile_pool(name="den", bufs=4) as den_p,
            tc.tile_pool(name="const", bufs=1) as const_p,
            tc.tile_pool(name="ro", bufs=4) as ro_p,
            tc.tile_pool(name="dram", bufs=1, space="DRAM") as dram_p,
        ):
            pools = {"psum": psum_p, "gather": gather_p, "idx": idx_p,
                     "dloc": dloc_p, "oh": oh_p, "stage": stage_p,
                     "ic": ic_p, "tmp": tmp_p, "ze": ze_p, "den": den_p,
                     "tmpb": tmpb_p, "zeb": zeb_p}
            iota_t = const_p.tile([128, 128], f32, tag="iota")
            nc.sync.dma_start(out=iota_t[:], in_=t["iota"][:])
            vrep_t = const_p.tile([128, D], f32, tag="vrep")
            nc.sync.dma_start(out=vrep_t[:], in_=t["vrep"][:])
            crep_t = const_p.tile([128, 1], f32, tag="crep")
            nc.sync.dma_start(out=crep_t[:], in_=t["crep"][:])

            x_loc = [dram_p.tile([128, nsub1, D], f32, tag="x_loc", name=f"x_loc{l}") for l in range(L)]
            x_full = [dram_p.tile([NC * 128, nsub1, D], f32, tag="x_full", name=f"x_full{l}")
                      for l in range(L - 1)]
            xbar_loc = dram_p.tile([128, nsub1, D], f32, tag="xbar_loc", name="xbar_loc")
            xbar_full = dram_p.tile([NC * 128, nsub1, D], f32, tag="xbar_full", name="xbar_full")
            rev_loc = dram_p.tile([128, nsub2, D], f32, tag="rev_loc", name="rev_loc")

            # ---- propagation layers ----
            for l in range(L):
                if l == 0:
                    src_view = t["emb"][:]
                    meta_l = meta1_l1
                    idx_l, dl_l = t["idx_l1"], t["dl_l1"]
                else:
                    src_view = x_full[l - 1][:].rearrange("a w d -> (a w) d")
                    meta_l = meta1_lx
                    idx_l, dl_l = t["idx_lx"], t["dl_lx"]
                _emit_phase(nc, tile, pools, meta_l, src_view,
                            idx_l[:], dl_l[:], x_loc[l],
                            invcnt_t=t["inv1"][:], iota_t=iota_t,
                            qstate=qstate, D=D)
                if l < L - 1:
                    nc.gpsimd.collective_compute(
                        "AllGather", mybir.AluOpType.bypass,
                        replica_groups=[list(range(NC))],
                        ins=[x_loc[l].opt()], outs=[x_full[l].opt()])

            # ---- readout mean ----
            RT = 16
            for w0 in range(0, nsub1, RT):
                wn = min(RT, nsub1 - w0)
                acc = ro_p.tile([128, RT, D], f32, tag="roacc")
                nc.sync.dma_start(out=acc[:, :wn, :],
                                  in_=t["emb_local"][:, w0:w0 + wn, :])
                for l in range(L):
                    tl = ro_p.tile([128, RT, D], f32, tag="rold")
                    nc.sync.dma_start(out=tl[:, :wn, :],
                                      in_=x_loc[l][:, w0:w0 + wn, :])
                    nc.vector.tensor_tensor(out=acc[:, :wn, :],
                                            in0=acc[:, :wn, :],
                                            in1=tl[:, :wn, :],
                                            op=mybir.AluOpType.add)
                nc.vector.tensor_scalar(out=acc[:, :wn, :], in0=acc[:, :wn, :],
                                        scalar1=1.0 / (L + 1), scalar2=None,
                                        op0=mybir.AluOpType.mult)
                nc.sync.dma_start(out=xbar_loc[:, w0:w0 + wn, :],
                                  in_=acc[:, :wn, :])
            nc.gpsimd.collective_compute(
                "AllGather", mybir.AluOpType.bypass,
                replica_groups=[list(range(NC))],
                ins=[xbar_loc.opt()], outs=[xbar_full.opt()])

            # ---- e2: review representations ----
            _emit_phase(nc, tile, pools, meta2,
                        xbar_full[:].rearrange("a w d -> (a w) d"),
                        t["idx_e2"][:], t["dl_e2"][:], rev_loc,
                        invcnt_t=t["inv2"][:], iota_t=iota_t,
                        qstate=qstate, D=D)

            if CFG.get("DEBUG"):
                for nm, srcv in (("d_x1", x_loc[0]), ("d_x2", x_loc[1]),
                                 ("d_x3", x_loc[2]), ("d_xbar", xbar_loc),
                                 ("d_rev", rev_loc)):
                    nsb = srcv.shape[1]
                    for w0 in range(0, nsb, 16):
                        wn = min(16, nsb - w0)
                        bt = ro_p.tile([128, 16, D], f32, tag="dbgb",
                                       name=f"dbgb_{nm}_{w0}")
                        nc.sync.dma_start(out=bt[:, :wn, :],
                                          in_=srcv[:, w0:w0 + wn, :])
                        nc.sync.dma_start(out=dbg_t[nm][:, w0:w0 + wn, :],
                                          in_=bt[:, :wn, :])

            # ---- e3: edge-softmax attention ----
            _emit_phase(nc, tile, pools, meta3,
                        rev_loc[:].rearrange("p w d -> (p w) d"),
                        t["idx_e3"][:], t["dl_e3"][:], out_t,
                        invcnt_t=None, iota_t=iota_t,
                        e3=(vrep_t, crep_t), qstate=qstate, D=D)

    nc.compile()

    res = run_bass_kernel_spmd(nc, in_maps, core_ids=list(range(NC)),
                               trace=CFG["TRACE"] or os.environ.get("GNN_TRACE") == "1")
    _LAST["exec_ns"] = res.exec_time_ns
    _LAST["profile_json"] = res.profile_json
    _LAST["results"] = res.results

    out = np.empty((M, D), np.float32)
    for i in range(NC):
        o = res.results[i]["out"]          # [128, nsub3, D]
        r = np.arange(MSH)
        out[i * MSH:(i + 1) * MSH] = o[r % 128, r // 128]
    return out



# revision 2
# speedup vs baseline: 1.1497x; 1.1497x over previous
"""LightGCN-style GNN on 8 Trainium2 NeuronCores — v2.

Differences from v1 (baseline):
  * Tables stored as duplicated bf16 rows [rows, 128] (row = [x, x]); SWDGE
    gathers fetch 256B bf16 elements directly usable as matmul rhs — the
    per-piece f32->bf16 CAST pass (5.8ms of DVE time in v1) is gone.
  * Gathers use prepare_only + trigger_dma so GPSIMD descriptor generation
    pipelines with DMA transfers across the 4 SWDGE queues.
  * Slot packing alignment is configurable (ALIGN=0 packs cells tightly;
    blocks may straddle subwindows, handled by offset-adjusted dloc values
    and per-block subwindow spans).
  * One-hot scatter matrices built from fp16 iota/dloc (2x DVE rate); PSUM
    accumulation uses static start/stop flags (no bank memsets).
  * e3 attention folds the softmax denominator into the feature matmul via
    a ones-column in the review table (col 64).
"""

import os
import sys
import types

import numpy as np
import ml_dtypes

BF16 = ml_dtypes.bfloat16

CFG = {
    "R": 400_000,
    "M": 100_000,
    "L": 3,
    "NC": 8,
    "CH": 32768,
    "NSUP": 16,       # subwindows per superwindow (e1/e2)
    "NSUP3": 8,       # subwindows per superwindow (e3)
    "NQ": 4,
    "OHG": 8,         # one-hot build group (blocks per DVE op)
    "ALIGN": 0,       # 0 = tight cell packing, 128 = per-cell alignment
    "SP": False,      # single_packet (True crashes HW at ~4k descriptors)
    "PREP": False,    # prepare_only + trigger_dma pipelining
    "TRACE": False,
}

_LAST = {"exec_ns": None, "profile_json": None}


def _install_profile_hook():
    try:
        if "antenv.axon_hooks" in sys.modules:
            return
        import antenv

        mod = types.ModuleType("antenv.axon_hooks")
        mod._hook = None
        mod.set_axon_ntff_profile_hook = lambda h: setattr(mod, "_hook", h)
        mod.get_axon_ntff_profile_hook = lambda: mod._hook
        sys.modules["antenv.axon_hooks"] = mod
        antenv.axon_hooks = mod
        from trn_agent_boot.trn_boot import _ntff_profile_via_ctypes

        mod.set_axon_ntff_profile_hook(
            _ntff_profile_via_ctypes("/opt/axon/libaxon_pjrt.so")
        )
    except Exception:
        pass


def _ru(x, m):
    return -(-np.asarray(x) // m) * m


# ---------------------------------------------------------------------------
# host-side slot structure
# ---------------------------------------------------------------------------
class Meta:
    """Uniform (cross-core) slot structure of one gather/reduce phase."""

    def __init__(self, percore, nsub, nsup, table_rows):
        CH = CFG["CH"]
        ncore = len(percore)
        nchunk = -(-table_rows // CH)
        self.nsub, self.nsup, self.nchunk = nsub, nsup, nchunk
        self.table_rows = table_rows
        self.nsuper = nsub // nsup
        npiece = self.nsuper * nchunk
        self.npiece = npiece

        cell = np.zeros((ncore, npiece * nsup), np.int64)
        for i, (dl, sf) in enumerate(percore):
            w = dl >> 7
            s = w // nsup
            wl = w - s * nsup
            c = sf // CH
            cell[i] = np.bincount((s * nchunk + c) * nsup + wl,
                                  minlength=npiece * nsup)
        cellcnt = cell.reshape(ncore, npiece, nsup)
        align = CFG["ALIGN"]
        if align:
            cellcap = _ru(cellcnt.max(0), align)          # [npiece, nsup]
            piece_cap = _ru(cellcap.sum(1), 128)
            wl_off = np.broadcast_to((np.cumsum(cellcap, 1) - cellcap)[None],
                                     (ncore, npiece, nsup)).copy()
            span_lo = wl_off[0].copy()
            span_hi = wl_off[0] + cellcap
        else:
            piece_cap = _ru(cellcnt.sum(2).max(0), 128)   # [npiece]
            wl_off = np.cumsum(cellcnt, 2) - cellcnt       # per-core
            span_lo = wl_off.min(0)
            span_hi = (wl_off + cellcnt).max(0)
        has = cellcnt.max(0) > 0                           # [npiece, nsup]
        self.cellcnt = cellcnt
        self.wl_off = wl_off
        self.piece_cap = piece_cap.astype(np.int64)
        self.piece_base = np.concatenate(
            [[0], np.cumsum(self.piece_cap)])[:-1].astype(np.int64)
        self.total = int(self.piece_cap.sum())

        # static per-piece block schedule: blocks[p] = [(wl_base, [wl,...])]
        self.blocks = []
        for p in range(npiece):
            nb = int(self.piece_cap[p]) // 128
            bl = []
            for b in range(nb):
                lo, hi = 128 * b, 128 * (b + 1)
                wls = [wl for wl in range(nsup)
                       if has[p, wl] and span_lo[p, wl] < hi
                       and span_hi[p, wl] > lo]
                bl.append((wls[0] if wls else 0, wls))
            self.blocks.append(bl)

        # per-(super, wl) total matmul count (for start/stop flags)
        self.mm_total = np.zeros((self.nsuper, nsup), np.int64)
        for p in range(npiece):
            s = p // nchunk
            for _, wls in self.blocks[p]:
                for wl in wls:
                    self.mm_total[s, wl] += 1
        # subwindow-has-any-edge (for stage memsets)
        self.w_has = (cellcnt.sum(2) if False else
                      cellcnt.max(0).reshape(npiece, nsup))
        wh = np.zeros(nsub, bool)
        for p in range(npiece):
            s = p // nchunk
            wh[s * nsup:(s + 1) * nsup] |= has[p]
        self.w_has_edges = wh

        # per-block wl_base lookup for slot packing: [total/128]
        self.blk_base = np.zeros(self.total // 128, np.int64)
        for p in range(npiece):
            b0 = int(self.piece_base[p]) // 128
            for b, (wb, _) in enumerate(self.blocks[p]):
                self.blk_base[b0 + b] = wb

    def pack(self, core, dl, sf):
        """Per-core slot assignment -> (idx16 [128, T/16], dloc [128, T/128])."""
        CH = CFG["CH"]
        nsup, nchunk = self.nsup, self.nchunk
        T = self.total
        idxval = np.zeros(T, np.int16)
        dval = np.full(T, -1.0, np.float16)
        if len(dl):
            w = dl >> 7
            s = w // nsup
            wl = w - s * nsup
            c = sf // CH
            p = s * nchunk + c
            key = p * nsup + wl
            order = np.argsort(key, kind="stable")
            ks = key[order]
            starts = np.flatnonzero(np.concatenate([[True], ks[1:] != ks[:-1]]))
            rank = (np.arange(len(ks))
                    - np.repeat(starts, np.diff(np.append(starts, len(ks)))))
            po, wlo = p[order], wl[order]
            slot = (self.piece_base[po] + self.wl_off[core, po, wlo] + rank)
            idxval[slot] = (sf[order] & (CH - 1)).astype(np.int16)
            dsl = dl[order] - (s[order] * nsup) * 128        # super-local
            dval[slot] = (dsl - 128 * self.blk_base[slot >> 7]).astype(np.float16)
        m = idxval.reshape(T // 16, 16).T
        idx16 = np.tile(m, (8, 1))                           # [128, T/16]
        dloc = dval.reshape(T // 128, 128).T.copy()          # [128, T/128]
        return idx16, dloc


def _invcnt_pmajor(dstloc, nsub):
    cnt = np.bincount(dstloc, minlength=nsub * 128)
    inv = 1.0 / np.maximum(cnt, 1)
    return inv.reshape(nsub, 128).T.astype(np.float32).copy()


# ---------------------------------------------------------------------------
# device phase emitter
# ---------------------------------------------------------------------------
def _emit_phase(nc, pools, meta, src_view, idx_t, dloc_t, out_tile,
                invcnt_t=None, iota_t=None, e3=None, qstate=None, qsems=None,
                dup_out=True, ones_col=False):
    import concourse.mybir as mybir

    f32 = mybir.dt.float32
    bf16 = mybir.dt.bfloat16
    fp16 = mybir.dt.float16
    CH = CFG["CH"]
    nsup = meta.nsup
    nchunk = meta.nchunk
    OHG = CFG["OHG"]
    slotw = 64 if e3 is None else 65
    spb = 512 // slotw
    nbanks = -(-nsup // spb)

    for s in range(meta.nsuper):
        banks = [pools["psum"].tile([128, 512], f32, tag="bank",
                                    name=f"bank{bi}") for bi in range(nbanks)]
        for bk in banks:
            nc.vector.memset(bk[:], 0.0)

        def bank_slice(wl, lo, hi):
            return banks[wl // spb][:, (wl % spb) * slotw + lo:
                                    (wl % spb) * slotw + hi]

        seen = {}
        for c in range(nchunk):
            p = s * nchunk + c
            cap = int(meta.piece_cap[p])
            if cap == 0:
                continue
            A = cap // 128
            base = int(meta.piece_base[p])
            it = pools["idx"].tile([128, cap // 16], mybir.dt.int16, tag="idx")
            nc.sync.dma_start(out=it[:],
                              in_=idx_t[:, base // 16:(base + cap) // 16])
            dlt = pools["dloc"].tile([128, A], fp16, tag="dloc")
            nc.sync.dma_start(out=dlt[:],
                              in_=dloc_t[:, base // 128:base // 128 + A])
            gt = pools["gather"].tile([128, A, 128], bf16, tag="gt")
            lo, hi = c * CH, min((c + 1) * CH, meta.table_rows)
            q = qstate[0] % CFG["NQ"]
            qstate[0] += 1
            if CFG["PREP"]:
                nc.gpsimd.dma_gather(
                    out_ap=gt[:], in_ap=src_view[lo:hi], idxs_ap=it[:],
                    num_idxs=cap, num_idxs_reg=cap, elem_size=128,
                    queue_num=q, single_packet=CFG["SP"],
                    prepare_only=True, sem=qsems[q])
                nc.gpsimd.trigger_dma(count=None, queue_num=q)
            else:
                nc.gpsimd.dma_gather(
                    out_ap=gt[:], in_ap=src_view[lo:hi], idxs_ap=it[:],
                    num_idxs=cap, num_idxs_reg=cap, elem_size=128,
                    queue_num=q, single_packet=CFG["SP"])

            if e3 is not None:
                vrep, crep = e3
                tmp = pools["tmp"].tile([128, A, 64], f32, tag="tmp")
                nc.vector.tensor_tensor(
                    out=tmp[:], in0=gt[:, :, 0:64],
                    in1=vrep[:].rearrange("p (o d) -> p o d", o=1)
                        .to_broadcast([128, A, 64]),
                    op=mybir.AluOpType.mult)
                ze = pools["ze"].tile([128, A], f32, tag="ze")
                nc.vector.tensor_reduce(out=ze[:], in_=tmp[:],
                                        axis=mybir.AxisListType.X,
                                        op=mybir.AluOpType.add)
                nc.scalar.activation(out=ze[:], in_=ze[:],
                                     func=mybir.ActivationFunctionType.Exp,
                                     bias=crep[:, 0:1], scale=1.0)
                rhs = pools["rhs"].tile([128, A, 65], bf16, tag="rhs")
                nc.vector.tensor_tensor(
                    out=rhs[:], in0=gt[:, :, 0:65],
                    in1=ze[:].rearrange("p (a o) -> p a o", o=1)
                        .to_broadcast([128, A, 65]),
                    op=mybir.AluOpType.mult)
                rhs_w = 65
            else:
                rhs = gt
                rhs_w = 64

            blocks = meta.blocks[p]

            def emit_mms(b, oh_ap, wb, wls):
                for wl in wls:
                    k = seen.get(wl, 0)
                    seen[wl] = k + 1
                    nc.tensor.matmul(
                        out=bank_slice(wl, 0, slotw),
                        lhsT=oh_ap[:, (wl - wb) * 128:(wl - wb + 1) * 128],
                        rhs=rhs[:, b, 0:rhs_w],
                        start=False,
                        stop=(k + 1 == meta.mm_total[s, wl]),
                        skip_group_check=True)

            b = 0
            while b < len(blocks):
                wb, wls = blocks[b]
                span = (wls[-1] - wb + 1) if wls else 1
                if span <= 2:
                    g = 1
                    while (g < OHG and b + g < len(blocks)):
                        wb2, wls2 = blocks[b + g]
                        if wls2 and wls2[-1] - wb2 + 1 > 2:
                            break
                        g += 1
                    oh = pools["oh"].tile([128, OHG, 256], bf16, tag="oh")
                    nc.vector.tensor_tensor(
                        out=oh[:, :g, :],
                        in0=iota_t[:, 0:256]
                            .rearrange("p (o x) -> p o x", o=1)
                            .to_broadcast([128, g, 256]),
                        in1=dlt[:, b:b + g]
                            .rearrange("p (a o) -> p a o", o=1)
                            .to_broadcast([128, g, 256]),
                        op=mybir.AluOpType.is_equal)
                    for j in range(g):
                        wbj, wlsj = blocks[b + j]
                        emit_mms(b + j, oh[:, j, :], wbj, wlsj)
                    b += g
                else:
                    oh = pools["oh3"].tile([128, span * 128], bf16, tag="oh3")
                    nc.vector.tensor_tensor(
                        out=oh[:],
                        in0=iota_t[:, 0:span * 128],
                        in1=dlt[:, b:b + 1].to_broadcast([128, span * 128]),
                        op=mybir.AluOpType.is_equal)
                    emit_mms(b, oh[:], wb, wls)
                    b += 1

        # ---- normalize + stage out ----
        if e3 is None:
            stage = pools["stage"].tile([128, nsup, 128],
                                        mybir.dt.bfloat16, tag="stage")
            if invcnt_t is not None:
                ic = pools["ic"].tile([128, nsup], f32, tag="ic")
                nc.sync.dma_start(out=ic[:],
                                  in_=invcnt_t[:, s * nsup:(s + 1) * nsup])
            for wl in range(nsup):
                w = s * nsup + wl
                if not meta.w_has_edges[w]:
                    nc.vector.memset(stage[:, wl, :], 0.0)
                    continue
                for half in (0, 1):
                    nc.vector.tensor_scalar(
                        out=stage[:, wl, half * 64:half * 64 + 64],
                        in0=bank_slice(wl, 0, 64),
                        scalar1=ic[:, wl:wl + 1], scalar2=None,
                        op0=mybir.AluOpType.mult)
            if ones_col:
                nc.vector.memset(stage[:, :, 64:65], 1.0)
            nc.sync.dma_start(out=out_tile[:, s * nsup:(s + 1) * nsup, :],
                              in_=stage[:])
        else:
            stage = pools["stage3"].tile([128, nsup, 64], f32, tag="stage3")
            for wl in range(nsup):
                w = s * nsup + wl
                if not meta.w_has_edges[w]:
                    nc.vector.memset(stage[:, wl, :], 0.0)
                    continue
                dt = pools["den"].tile([128, 1], f32, tag="den")
                nc.vector.tensor_scalar(
                    out=dt[:], in0=bank_slice(wl, 64, 65),
                    scalar1=1e-9, scalar2=None, op0=mybir.AluOpType.max)
                nc.vector.reciprocal(out=dt[:], in_=dt[:])
                nc.vector.tensor_scalar(
                    out=stage[:, wl, :], in0=bank_slice(wl, 0, 64),
                    scalar1=dt[:, 0:1], scalar2=None,
                    op0=mybir.AluOpType.mult)
            nc.sync.dma_start(out=out_tile[:, s * nsup:(s + 1) * nsup, :],
                              in_=stage[:])


# ---------------------------------------------------------------------------
# main kernel
# ---------------------------------------------------------------------------
def kernel(**inputs):
    _install_profile_hook()
    import concourse.bacc as bacc
    import concourse.mybir as mybir
    import concourse.tile as tile
    from concourse.bass_utils import run_bass_kernel_spmd

    f32 = mybir.dt.float32
    bf16 = mybir.dt.bfloat16
    fp16 = mybir.dt.float16

    emb = np.asarray(inputs["emb_table"], np.float32)
    node_ids = np.asarray(inputs["node_ids"])
    w_o = np.asarray(inputs["w_o"], np.float32)
    b_o = np.asarray(inputs["b_o"], np.float32)
    att_w = np.asarray(inputs["att_w"], np.float32)
    att_b = np.asarray(inputs["att_b"], np.float32)
    e1_src = np.asarray(inputs["e1_src"], np.int64)
    e1_dst = np.asarray(inputs["e1_dst"], np.int64)
    e2_src = np.asarray(inputs["e2_src"], np.int64)
    e2_dst = np.asarray(inputs["e2_dst"], np.int64)
    e3_src = np.asarray(inputs["e3_src"], np.int64)
    e3_dst = np.asarray(inputs["e3_dst"], np.int64)

    N, D = emb.shape
    R, M, L = CFG["R"], CFG["M"], CFG["L"]
    NC, CH, NSUP, NSUP3 = CFG["NC"], CFG["CH"], CFG["NSUP"], CFG["NSUP3"]
    RUN = CFG.get("RUN", True)

    x0 = emb[node_ids]
    x0b = x0.astype(BF16)
    emb_dup = np.concatenate([x0b, x0b], axis=1)            # [N, 128]
    v = (w_o @ att_w).astype(np.float32).ravel()
    c_sc = float(b_o @ att_w.ravel() + att_b.ravel()[0])

    NSH = N // NC
    MSH = M // NC
    nsub1 = int(_ru(-(-NSH // 128), NSUP))
    rows_x = NC * 128 * nsub1
    nsub3 = int(_ru(-(-MSH // 128), NSUP3))

    # ---------------- e1 edges per core (dst-range shard) -----------------
    core_of1 = np.minimum(e1_dst // NSH, NC - 1)
    e1_by_core = []
    for i in range(NC):
        m = core_of1 == i
        e1_by_core.append((e1_dst[m] - i * NSH, e1_src[m]))
    meta1_l1 = Meta(e1_by_core, nsub1, NSUP, N)
    e1_by_core_pm = []
    for d, s in e1_by_core:
        ci = np.minimum(s // NSH, NC - 1)
        r = s - ci * NSH
        flat = (ci * 128 + (r % 128)) * nsub1 + r // 128
        e1_by_core_pm.append((d, flat))
    meta1_lx = Meta(e1_by_core_pm, nsub1, NSUP, rows_x)

    # ---------------- e2: consumer-sharded reviews ------------------------
    e2cnt = np.bincount(e2_dst, minlength=R)
    core_of3 = np.minimum(e3_dst // MSH, NC - 1)
    ci2 = np.minimum(e2_src // NSH, NC - 1)
    r2 = e2_src - ci2 * NSH
    e2_srcflat = (ci2 * 128 + (r2 % 128)) * nsub1 + r2 // 128
    e2_chunk = e2_srcflat // CH

    o2 = np.lexsort((e2_chunk, e2_dst))
    e2d_s, e2c_s = e2_dst[o2], e2_chunk[o2]
    rstart = np.searchsorted(e2d_s, np.arange(R + 1))
    cmin = np.full(R, 99, np.int64)
    cmax = np.full(R, 99, np.int64)
    hasr = rstart[1:] > rstart[:-1]
    if len(e2c_s):
        cmin[hasr] = e2c_s[rstart[:-1][hasr]]
        cmax[hasr] = e2c_s[rstart[1:][hasr] - 1]

    cons_lists, e2_data, e3_data, inv2_list = [], [], [], []
    for i in range(NC):
        m3 = core_of3 == i
        src3 = e3_src[m3]
        dst3 = e3_dst[m3] - i * MSH
        cons = np.unique(src3)
        key = cmin[cons].astype(np.int64) * 100 + cmax[cons]
        cons = cons[np.argsort(key, kind="stable")]
        lid = np.full(R, -1, np.int64)
        lid[cons] = np.arange(len(cons))
        cons_lists.append(cons)
        sel = lid[e2_dst] >= 0
        e2_data.append((lid[e2_dst[sel]], e2_srcflat[sel]))
        e3_data.append((dst3, lid[src3]))
        inv2_list.append((1.0 / np.maximum(e2cnt[cons], 1)).astype(np.float32))

    revcap = max(len(c) for c in cons_lists)
    nsub2 = int(_ru(-(-revcap // 128), NSUP))
    rows_rev = 128 * nsub2
    meta2 = Meta(e2_data, nsub2, NSUP, rows_x)

    def map_rev(r):
        return (r % 128) * nsub2 + (r // 128)

    e3_data_pm = [(d, map_rev(s)) for d, s in e3_data]
    meta3 = Meta(e3_data_pm, nsub3, NSUP3, rows_rev)

    # ---------------- per-core input arrays -------------------------------
    iota = np.zeros((128, 2048), np.float16)
    iota[:] = np.arange(2048).astype(np.float16)[None, :]

    in_maps = []
    for i in range(NC):
        d1, s1 = e1_by_core[i]
        idxA, dlA = meta1_l1.pack(i, d1, s1)
        d1x, s1x = e1_by_core_pm[i]
        idxB, dlB = meta1_lx.pack(i, d1x, s1x)
        inv1 = _invcnt_pmajor(d1, nsub1)
        d2, s2 = e2_data[i]
        idx2, dl2 = meta2.pack(i, d2, s2)
        inv2 = np.zeros((128, nsub2), np.float32)
        li = np.arange(len(cons_lists[i]))
        inv2[li % 128, li // 128] = inv2_list[i]
        d3, s3 = e3_data_pm[i]
        idx3, dl3 = meta3.pack(i, d3, s3)
        embl = np.zeros((128, nsub1, 128), BF16)
        loc = x0b[i * NSH:(i + 1) * NSH]
        r = np.arange(NSH)
        embl[r % 128, r // 128, 0:64] = loc
        embl[r % 128, r // 128, 64:128] = loc
        in_maps.append({
            "emb": emb_dup,
            "emb_local": embl,
            "idx_l1": idxA, "dl_l1": dlA,
            "idx_lx": idxB, "dl_lx": dlB,
            "inv1": inv1,
            "idx_e2": idx2, "dl_e2": dl2, "inv2": inv2,
            "idx_e3": idx3, "dl_e3": dl3,
            "iota": iota,
            "vrep": np.tile(v, (128, 1)).astype(np.float32),
            "crep": np.full((128, 1), c_sc, np.float32),
        })

    # ---------------- build device program --------------------------------
    nc = bacc.Bacc("TRN2", target_bir_lowering=False, debug=False,
                   num_devices=NC, num_swdge_queues=CFG["NQ"])

    def din(name, arr):
        return nc.dram_tensor(name, list(arr.shape),
                              mybir.dt.from_np(arr.dtype), kind="ExternalInput")

    t = {k: din(k, in_maps[0][k]) for k in in_maps[0]}
    out_t = nc.dram_tensor("out", [128, nsub3, 64], f32, kind="ExternalOutput")
    dbg_t = {}
    if CFG.get("DEBUG"):
        for nm, sh in (("d_x1", [128, nsub1, 128]), ("d_x2", [128, nsub1, 128]),
                       ("d_x3", [128, nsub1, 128]), ("d_xbar", [128, nsub1, 128]),
                       ("d_rev", [128, nsub2, 128])):
            dbg_t[nm] = nc.dram_tensor(nm, sh, bf16, kind="ExternalOutput")

    qstate = [0]
    with tile.TileContext(nc) as tc:
        qsems = [nc.alloc_semaphore(f"gsem{q}") for q in range(CFG["NQ"])]
        with (
            tc.tile_pool(name="psum", bufs=6, space="PSUM") as psum_p,
            tc.tile_pool(name="gather", bufs=6) as gather_p,
            tc.tile_pool(name="idx", bufs=6) as idx_p,
            tc.tile_pool(name="dloc", bufs=6) as dloc_p,
            tc.tile_pool(name="oh", bufs=4) as oh_p,
            tc.tile_pool(name="oh3", bufs=2) as oh3_p,
            tc.tile_pool(name="stage", bufs=3) as stage_p,
            tc.tile_pool(name="stage3", bufs=2) as stage3_p,
            tc.tile_pool(name="ic", bufs=3) as ic_p,
            tc.tile_pool(name="tmp", bufs=3) as tmp_p,
            tc.tile_pool(name="rhs", bufs=3) as rhs_p,
            tc.tile_pool(name="ze", bufs=3) as ze_p,
            tc.tile_pool(name="den", bufs=4) as den_p,
            tc.tile_pool(name="const", bufs=1) as const_p,
            tc.tile_pool(name="ro", bufs=3) as ro_p,
            tc.tile_pool(name="dram", bufs=1, space="DRAM") as dram_p,
        ):
            pools = {"psum": psum_p, "gather": gather_p, "idx": idx_p,
                     "dloc": dloc_p, "oh": oh_p, "oh3": oh3_p,
                     "stage": stage_p, "stage3": stage3_p, "ic": ic_p,
                     "tmp": tmp_p, "rhs": rhs_p, "ze": ze_p, "den": den_p}
            iota_t = const_p.tile([128, 2048], fp16, tag="iota")
            nc.sync.dma_start(out=iota_t[:], in_=t["iota"][:])
            vrep_t = const_p.tile([128, 64], f32, tag="vrep")
            nc.sync.dma_start(out=vrep_t[:], in_=t["vrep"][:])
            crep_t = const_p.tile([128, 1], f32, tag="crep")
            nc.sync.dma_start(out=crep_t[:], in_=t["crep"][:])

            x_loc = [dram_p.tile([128, nsub1, 128], bf16, tag="x_loc",
                                 name=f"x_loc{l}") for l in range(L)]
            x_full = [dram_p.tile([NC * 128, nsub1, 128], bf16, tag="x_full",
                                  name=f"x_full{l}") for l in range(L - 1)]
            xbar_loc = dram_p.tile([128, nsub1, 128], bf16, tag="xbar_loc",
                                   name="xbar_loc")
            xbar_full = dram_p.tile([NC * 128, nsub1, 128], bf16,
                                    tag="xbar_full", name="xbar_full")
            rev_loc = dram_p.tile([128, nsub2, 128], bf16, tag="rev_loc",
                                  name="rev_loc")

            # ---- propagation layers ----
            for l in range(L):
                if l == 0:
                    src_view = t["emb"][:]
                    meta_l = meta1_l1
                    idx_l, dl_l = t["idx_l1"], t["dl_l1"]
                else:
                    xf = x_loc[l - 1] if NC == 1 else x_full[l - 1]
                    src_view = xf[:].rearrange("a w d -> (a w) d")
                    meta_l = meta1_lx
                    idx_l, dl_l = t["idx_lx"], t["dl_lx"]
                _emit_phase(nc, pools, meta_l, src_view, idx_l[:], dl_l[:],
                            x_loc[l], invcnt_t=t["inv1"][:], iota_t=iota_t,
                            qstate=qstate, qsems=qsems)
                if l < L - 1 and NC > 1:
                    nc.gpsimd.collective_compute(
                        "AllGather", mybir.AluOpType.bypass,
                        replica_groups=[list(range(NC))],
                        ins=[x_loc[l].opt()], outs=[x_full[l].opt()])

            # ---- readout mean ----
            RT = 8
            for w0 in range(0, nsub1, RT):
                wn = min(RT, nsub1 - w0)
                t0 = ro_p.tile([128, RT, 128], bf16, tag="roemb")
                nc.sync.dma_start(out=t0[:, :wn, :],
                                  in_=t["emb_local"][:, w0:w0 + wn, :])
                acc = ro_p.tile([128, RT, 128], f32, tag="roacc")
                tl = ro_p.tile([128, RT, 128], bf16, tag="rold0")
                nc.sync.dma_start(out=tl[:, :wn, :],
                                  in_=x_loc[0][:, w0:w0 + wn, :])
                nc.vector.tensor_tensor(out=acc[:, :wn, :],
                                        in0=t0[:, :wn, :], in1=tl[:, :wn, :],
                                        op=mybir.AluOpType.add)
                for l in range(1, L):
                    tl = ro_p.tile([128, RT, 128], bf16, tag="rold")
                    nc.sync.dma_start(out=tl[:, :wn, :],
                                      in_=x_loc[l][:, w0:w0 + wn, :])
                    nc.vector.tensor_tensor(out=acc[:, :wn, :],
                                            in0=acc[:, :wn, :],
                                            in1=tl[:, :wn, :],
                                            op=mybir.AluOpType.add)
                accb = ro_p.tile([128, RT, 128], bf16, tag="roaccb")
                nc.vector.tensor_scalar(out=accb[:, :wn, :],
                                        in0=acc[:, :wn, :],
                                        scalar1=1.0 / (L + 1), scalar2=None,
                                        op0=mybir.AluOpType.mult)
                nc.sync.dma_start(out=xbar_loc[:, w0:w0 + wn, :],
                                  in_=accb[:, :wn, :])
            if NC > 1:
                nc.gpsimd.collective_compute(
                    "AllGather", mybir.AluOpType.bypass,
                    replica_groups=[list(range(NC))],
                    ins=[xbar_loc.opt()], outs=[xbar_full.opt()])

            # ---- e2: review representations (adds ones column) ----
            _emit_phase(nc, pools, meta2,
                        (xbar_loc if NC == 1 else xbar_full)[:]
                        .rearrange("a w d -> (a w) d"),
                        t["idx_e2"][:], t["dl_e2"][:], rev_loc,
                        invcnt_t=t["inv2"][:], iota_t=iota_t,
                        qstate=qstate, qsems=qsems, ones_col=True)

            if CFG.get("DEBUG"):
                for nm, srcv in (("d_x1", x_loc[0]), ("d_x2", x_loc[1]),
                                 ("d_x3", x_loc[2]), ("d_xbar", xbar_loc),
                                 ("d_rev", rev_loc)):
                    nsb = srcv.shape[1]
                    for w0 in range(0, nsb, 16):
                        wn = min(16, nsb - w0)
                        bt = stage_p.tile([128, 16, 128], bf16, tag="dbgb",
                                          name=f"dbgb_{nm}_{w0}")
                        nc.sync.dma_start(out=bt[:, :wn, :],
                                          in_=srcv[:, w0:w0 + wn, :])
                        nc.sync.dma_start(out=dbg_t[nm][:, w0:w0 + wn, :],
                                          in_=bt[:, :wn, :])

            # ---- e3: edge-softmax attention ----
            _emit_phase(nc, pools, meta3,
                        rev_loc[:].rearrange("p w d -> (p w) d"),
                        t["idx_e3"][:], t["dl_e3"][:], out_t,
                        invcnt_t=None, iota_t=iota_t,
                        e3=(vrep_t, crep_t), qstate=qstate, qsems=qsems)

    nc.compile()
    if not RUN:
        meta_out = dict(nsub3=nsub3, MSH=MSH)
        return nc, in_maps, meta_out

    res = run_bass_kernel_spmd(
        nc, in_maps, core_ids=list(range(NC)),
        trace=CFG["TRACE"] or os.environ.get("GNN_TRACE") == "1")
    _LAST["exec_ns"] = res.exec_time_ns
    _LAST["profile_json"] = res.profile_json
    _LAST["results"] = res.results

    out = np.empty((M, D), np.float32)
    for i in range(NC):
        o = res.results[i]["out"]
        r = np.arange(MSH)
        out[i * MSH:(i + 1) * MSH] = o[r % 128, r // 128]
    return out


# revision 3
# speedup vs baseline: 1.1710x; 1.0185x over previous
"""LightGCN-style GNN on 8 Trainium2 NeuronCores — v2.

Differences from v1 (baseline):
  * Tables stored as duplicated bf16 rows [rows, 128] (row = [x, x]); SWDGE
    gathers fetch 256B bf16 elements directly usable as matmul rhs — the
    per-piece f32->bf16 CAST pass (5.8ms of DVE time in v1) is gone.
  * Gathers use prepare_only + trigger_dma so GPSIMD descriptor generation
    pipelines with DMA transfers across the 4 SWDGE queues.
  * Slot packing alignment is configurable (ALIGN=0 packs cells tightly;
    blocks may straddle subwindows, handled by offset-adjusted dloc values
    and per-block subwindow spans).
  * One-hot scatter matrices built from fp16 iota/dloc (2x DVE rate); PSUM
    accumulation uses static start/stop flags (no bank memsets).
  * e3 attention folds the softmax denominator into the feature matmul via
    a ones-column in the review table (col 64).
"""

import os
import sys
import types

import numpy as np
import ml_dtypes

BF16 = ml_dtypes.bfloat16

CFG = {
    "R": 400_000,
    "M": 100_000,
    "L": 3,
    "NC": 8,
    "CH": 32768,
    "NSUP": 16,       # subwindows per superwindow (e1/e2)
    "NSUP3": 8,       # subwindows per superwindow (e3)
    "NQ": 4,
    "OHG": 8,         # one-hot build group (blocks per DVE op)
    "ALIGN": 0,       # 0 = tight cell packing, 128 = per-cell alignment
    "SP": False,      # single_packet (True crashes HW at ~4k descriptors)
    "PREP": False,    # prepare_only + trigger_dma pipelining
    "TRACE": False,
}

_LAST = {"exec_ns": None, "profile_json": None}


def _install_profile_hook():
    try:
        if "antenv.axon_hooks" in sys.modules:
            return
        import antenv

        mod = types.ModuleType("antenv.axon_hooks")
        mod._hook = None
        mod.set_axon_ntff_profile_hook = lambda h: setattr(mod, "_hook", h)
        mod.get_axon_ntff_profile_hook = lambda: mod._hook
        sys.modules["antenv.axon_hooks"] = mod
        antenv.axon_hooks = mod
        from trn_agent_boot.trn_boot import _ntff_profile_via_ctypes

        mod.set_axon_ntff_profile_hook(
            _ntff_profile_via_ctypes("/opt/axon/libaxon_pjrt.so")
        )
    except Exception:
        pass


def _ru(x, m):
    return -(-np.asarray(x) // m) * m


# ---------------------------------------------------------------------------
# host-side slot structure
# ---------------------------------------------------------------------------
class Meta:
    """Uniform (cross-core) slot structure of one gather/reduce phase."""

    def __init__(self, percore, nsub, nsup, table_rows):
        CH = CFG["CH"]
        ncore = len(percore)
        nchunk = -(-table_rows // CH)
        self.nsub, self.nsup, self.nchunk = nsub, nsup, nchunk
        self.table_rows = table_rows
        self.nsuper = nsub // nsup
        npiece = self.nsuper * nchunk
        self.npiece = npiece

        cell = np.zeros((ncore, npiece * nsup), np.int64)
        for i, (dl, sf) in enumerate(percore):
            w = dl >> 7
            s = w // nsup
            wl = w - s * nsup
            c = sf // CH
            cell[i] = np.bincount((s * nchunk + c) * nsup + wl,
                                  minlength=npiece * nsup)
        cellcnt = cell.reshape(ncore, npiece, nsup)
        align = CFG["ALIGN"]
        if align:
            cellcap = _ru(cellcnt.max(0), align)          # [npiece, nsup]
            piece_cap = _ru(cellcap.sum(1), 128)
            wl_off = np.broadcast_to((np.cumsum(cellcap, 1) - cellcap)[None],
                                     (ncore, npiece, nsup)).copy()
            span_lo = wl_off[0].copy()
            span_hi = wl_off[0] + cellcap
        else:
            piece_cap = _ru(cellcnt.sum(2).max(0), 128)   # [npiece]
            wl_off = np.cumsum(cellcnt, 2) - cellcnt       # per-core
            span_lo = wl_off.min(0)
            span_hi = (wl_off + cellcnt).max(0)
        has = cellcnt.max(0) > 0                           # [npiece, nsup]
        self.cellcnt = cellcnt
        self.wl_off = wl_off
        self.piece_cap = piece_cap.astype(np.int64)
        self.piece_base = np.concatenate(
            [[0], np.cumsum(self.piece_cap)])[:-1].astype(np.int64)
        self.total = int(self.piece_cap.sum())

        # static per-piece block schedule: blocks[p] = [(wl_base, [wl,...])]
        self.blocks = []
        for p in range(npiece):
            nb = int(self.piece_cap[p]) // 128
            bl = []
            for b in range(nb):
                lo, hi = 128 * b, 128 * (b + 1)
                wls = [wl for wl in range(nsup)
                       if has[p, wl] and span_lo[p, wl] < hi
                       and span_hi[p, wl] > lo]
                bl.append((wls[0] if wls else 0, wls))
            self.blocks.append(bl)

        # per-(super, wl) total matmul count (for start/stop flags)
        self.mm_total = np.zeros((self.nsuper, nsup), np.int64)
        for p in range(npiece):
            s = p // nchunk
            for _, wls in self.blocks[p]:
                for wl in wls:
                    self.mm_total[s, wl] += 1
        # subwindow-has-any-edge (for stage memsets)
        self.w_has = (cellcnt.sum(2) if False else
                      cellcnt.max(0).reshape(npiece, nsup))
        wh = np.zeros(nsub, bool)
        for p in range(npiece):
            s = p // nchunk
            wh[s * nsup:(s + 1) * nsup] |= has[p]
        self.w_has_edges = wh

        # per-block wl_base lookup for slot packing: [total/128]
        self.blk_base = np.zeros(self.total // 128, np.int64)
        for p in range(npiece):
            b0 = int(self.piece_base[p]) // 128
            for b, (wb, _) in enumerate(self.blocks[p]):
                self.blk_base[b0 + b] = wb

    def pack(self, core, dl, sf):
        """Per-core slot assignment -> (idx16 [128, T/16], dloc [128, T/128])."""
        CH = CFG["CH"]
        nsup, nchunk = self.nsup, self.nchunk
        T = self.total
        idxval = np.zeros(T, np.int16)
        dval = np.full(T, -1.0, np.float16)
        if len(dl):
            w = dl >> 7
            s = w // nsup
            wl = w - s * nsup
            c = sf // CH
            p = s * nchunk + c
            key = p * nsup + wl
            order = np.argsort(key, kind="stable")
            ks = key[order]
            starts = np.flatnonzero(np.concatenate([[True], ks[1:] != ks[:-1]]))
            rank = (np.arange(len(ks))
                    - np.repeat(starts, np.diff(np.append(starts, len(ks)))))
            po, wlo = p[order], wl[order]
            slot = (self.piece_base[po] + self.wl_off[core, po, wlo] + rank)
            idxval[slot] = (sf[order] & (CH - 1)).astype(np.int16)
            dsl = dl[order] - (s[order] * nsup) * 128        # super-local
            dval[slot] = (dsl - 128 * self.blk_base[slot >> 7]).astype(np.float16)
        m = idxval.reshape(T // 16, 16).T
        idx16 = np.tile(m, (8, 1))                           # [128, T/16]
        dloc = dval.reshape(T // 128, 128).T.copy()          # [128, T/128]
        return idx16, dloc


def _invcnt_pmajor(dstloc, nsub):
    cnt = np.bincount(dstloc, minlength=nsub * 128)
    inv = 1.0 / np.maximum(cnt, 1)
    return inv.reshape(nsub, 128).T.astype(np.float32).copy()


# ---------------------------------------------------------------------------
# device phase emitter
# ---------------------------------------------------------------------------
def _emit_phase(nc, pools, meta, src_view, idx_t, dloc_t, out_tile,
                invcnt_t=None, iota_t=None, e3=None, qstate=None, qsems=None,
                dup_out=True, ones_col=False, on_super=None, out_ap_fn=None):
    import concourse.mybir as mybir

    f32 = mybir.dt.float32
    bf16 = mybir.dt.bfloat16
    fp16 = mybir.dt.float16
    CH = CFG["CH"]
    nsup = meta.nsup
    nchunk = meta.nchunk
    OHG = CFG["OHG"]
    slotw = 64 if e3 is None else 65
    spb = 512 // slotw
    nbanks = -(-nsup // spb)

    for s in range(meta.nsuper):
        banks = [pools["psum"].tile([128, 512], f32, tag="bank",
                                    name=f"bank{bi}") for bi in range(nbanks)]
        for bk in banks:
            nc.vector.memset(bk[:], 0.0)

        def bank_slice(wl, lo, hi):
            return banks[wl // spb][:, (wl % spb) * slotw + lo:
                                    (wl % spb) * slotw + hi]

        seen = {}
        for c in range(nchunk):
            p = s * nchunk + c
            cap = int(meta.piece_cap[p])
            if cap == 0:
                continue
            A = cap // 128
            base = int(meta.piece_base[p])
            it = pools["idx"].tile([128, cap // 16], mybir.dt.int16, tag="idx")
            nc.sync.dma_start(out=it[:],
                              in_=idx_t[:, base // 16:(base + cap) // 16])
            dlt = pools["dloc"].tile([128, A], fp16, tag="dloc")
            nc.sync.dma_start(out=dlt[:],
                              in_=dloc_t[:, base // 128:base // 128 + A])
            gt = pools["gather"].tile([128, A, 128], bf16, tag="gt")
            lo, hi = c * CH, min((c + 1) * CH, meta.table_rows)
            q = qstate[0] % CFG["NQ"]
            qstate[0] += 1
            if CFG["PREP"]:
                nc.gpsimd.dma_gather(
                    out_ap=gt[:], in_ap=src_view[lo:hi], idxs_ap=it[:],
                    num_idxs=cap, num_idxs_reg=cap, elem_size=128,
                    queue_num=q, single_packet=CFG["SP"],
                    prepare_only=True, sem=qsems[q])
                nc.gpsimd.trigger_dma(count=None, queue_num=q)
            else:
                nc.gpsimd.dma_gather(
                    out_ap=gt[:], in_ap=src_view[lo:hi], idxs_ap=it[:],
                    num_idxs=cap, num_idxs_reg=cap, elem_size=128,
                    queue_num=q, single_packet=CFG["SP"])

            if e3 is not None:
                vrep, crep = e3
                tmp = pools["tmp"].tile([128, A, 64], f32, tag="tmp")
                nc.vector.tensor_tensor(
                    out=tmp[:], in0=gt[:, :, 0:64],
                    in1=vrep[:].rearrange("p (o d) -> p o d", o=1)
                        .to_broadcast([128, A, 64]),
                    op=mybir.AluOpType.mult)
                ze = pools["ze"].tile([128, A], f32, tag="ze")
                nc.vector.tensor_reduce(out=ze[:], in_=tmp[:],
                                        axis=mybir.AxisListType.X,
                                        op=mybir.AluOpType.add)
                nc.scalar.activation(out=ze[:], in_=ze[:],
                                     func=mybir.ActivationFunctionType.Exp,
                                     bias=crep[:, 0:1], scale=1.0)
                rhs = pools["rhs"].tile([128, A, 65], bf16, tag="rhs")
                nc.vector.tensor_tensor(
                    out=rhs[:], in0=gt[:, :, 0:65],
                    in1=ze[:].rearrange("p (a o) -> p a o", o=1)
                        .to_broadcast([128, A, 65]),
                    op=mybir.AluOpType.mult)
                rhs_w = 65
            else:
                rhs = gt
                rhs_w = 64

            blocks = meta.blocks[p]

            def emit_mms(b, oh_ap, wb, wls):
                for wl in wls:
                    k = seen.get(wl, 0)
                    seen[wl] = k + 1
                    nc.tensor.matmul(
                        out=bank_slice(wl, 0, slotw),
                        lhsT=oh_ap[:, (wl - wb) * 128:(wl - wb + 1) * 128],
                        rhs=rhs[:, b, 0:rhs_w],
                        start=False,
                        stop=(k + 1 == meta.mm_total[s, wl]),
                        skip_group_check=True)

            b = 0
            while b < len(blocks):
                wb, wls = blocks[b]
                span = (wls[-1] - wb + 1) if wls else 1
                if span <= 2:
                    g = 1
                    while (g < OHG and b + g < len(blocks)):
                        wb2, wls2 = blocks[b + g]
                        if wls2 and wls2[-1] - wb2 + 1 > 2:
                            break
                        g += 1
                    oh = pools["oh"].tile([128, OHG, 256], bf16, tag="oh")
                    nc.vector.tensor_tensor(
                        out=oh[:, :g, :],
                        in0=iota_t[:, 0:256]
                            .rearrange("p (o x) -> p o x", o=1)
                            .to_broadcast([128, g, 256]),
                        in1=dlt[:, b:b + g]
                            .rearrange("p (a o) -> p a o", o=1)
                            .to_broadcast([128, g, 256]),
                        op=mybir.AluOpType.is_equal)
                    for j in range(g):
                        wbj, wlsj = blocks[b + j]
                        emit_mms(b + j, oh[:, j, :], wbj, wlsj)
                    b += g
                else:
                    oh = pools["oh3"].tile([128, span * 128], bf16, tag="oh3")
                    nc.vector.tensor_tensor(
                        out=oh[:],
                        in0=iota_t[:, 0:span * 128],
                        in1=dlt[:, b:b + 1].to_broadcast([128, span * 128]),
                        op=mybir.AluOpType.is_equal)
                    emit_mms(b, oh[:], wb, wls)
                    b += 1

        # ---- normalize + stage out ----
        if e3 is None:
            stage = pools["stage"].tile([128, nsup, 128],
                                        mybir.dt.bfloat16, tag="stage")
            if invcnt_t is not None:
                ic = pools["ic"].tile([128, nsup], f32, tag="ic")
                nc.sync.dma_start(out=ic[:],
                                  in_=invcnt_t[:, s * nsup:(s + 1) * nsup])
            for wl in range(nsup):
                w = s * nsup + wl
                if not meta.w_has_edges[w]:
                    nc.vector.memset(stage[:, wl, :], 0.0)
                    continue
                for half in (0, 1):
                    nc.vector.tensor_scalar(
                        out=stage[:, wl, half * 64:half * 64 + 64],
                        in0=bank_slice(wl, 0, 64),
                        scalar1=ic[:, wl:wl + 1], scalar2=None,
                        op0=mybir.AluOpType.mult)
            if ones_col:
                nc.vector.memset(stage[:, :, 64:65], 1.0)
            oap = (out_ap_fn(s) if out_ap_fn is not None
                   else out_tile[:, s * nsup:(s + 1) * nsup, :])
            nc.sync.dma_start(out=oap, in_=stage[:])
            if on_super is not None:
                on_super(s)
        else:
            stage = pools["stage3"].tile([128, nsup, 64], f32, tag="stage3")
            for wl in range(nsup):
                w = s * nsup + wl
                if not meta.w_has_edges[w]:
                    nc.vector.memset(stage[:, wl, :], 0.0)
                    continue
                dt = pools["den"].tile([128, 1], f32, tag="den")
                nc.vector.tensor_scalar(
                    out=dt[:], in0=bank_slice(wl, 64, 65),
                    scalar1=1e-9, scalar2=None, op0=mybir.AluOpType.max)
                nc.vector.reciprocal(out=dt[:], in_=dt[:])
                nc.vector.tensor_scalar(
                    out=stage[:, wl, :], in0=bank_slice(wl, 0, 64),
                    scalar1=dt[:, 0:1], scalar2=None,
                    op0=mybir.AluOpType.mult)
            nc.sync.dma_start(out=out_tile[:, s * nsup:(s + 1) * nsup, :],
                              in_=stage[:])


# ---------------------------------------------------------------------------
# main kernel
# ---------------------------------------------------------------------------
def kernel(**inputs):
    _install_profile_hook()
    import concourse.bacc as bacc
    import concourse.mybir as mybir
    import concourse.tile as tile
    from concourse.bass_utils import run_bass_kernel_spmd

    f32 = mybir.dt.float32
    bf16 = mybir.dt.bfloat16
    fp16 = mybir.dt.float16

    emb = np.asarray(inputs["emb_table"], np.float32)
    node_ids = np.asarray(inputs["node_ids"])
    w_o = np.asarray(inputs["w_o"], np.float32)
    b_o = np.asarray(inputs["b_o"], np.float32)
    att_w = np.asarray(inputs["att_w"], np.float32)
    att_b = np.asarray(inputs["att_b"], np.float32)
    e1_src = np.asarray(inputs["e1_src"], np.int64)
    e1_dst = np.asarray(inputs["e1_dst"], np.int64)
    e2_src = np.asarray(inputs["e2_src"], np.int64)
    e2_dst = np.asarray(inputs["e2_dst"], np.int64)
    e3_src = np.asarray(inputs["e3_src"], np.int64)
    e3_dst = np.asarray(inputs["e3_dst"], np.int64)

    N, D = emb.shape
    R, M, L = CFG["R"], CFG["M"], CFG["L"]
    NC, CH, NSUP, NSUP3 = CFG["NC"], CFG["CH"], CFG["NSUP"], CFG["NSUP3"]
    RUN = CFG.get("RUN", True)

    x0 = emb[node_ids]
    x0b = x0.astype(BF16)
    emb_dup = np.concatenate([x0b, x0b], axis=1)            # [N, 128]
    v = (w_o @ att_w).astype(np.float32).ravel()
    c_sc = float(b_o @ att_w.ravel() + att_b.ravel()[0])

    NSH = N // NC
    MSH = M // NC
    nsub1 = int(_ru(-(-NSH // 128), NSUP))
    rows_x = NC * 128 * nsub1
    nsub3 = int(_ru(-(-MSH // 128), NSUP3))

    # ---------------- e1 edges per core (dst-range shard) -----------------
    core_of1 = np.minimum(e1_dst // NSH, NC - 1)
    e1_by_core = []
    for i in range(NC):
        m = core_of1 == i
        e1_by_core.append((e1_dst[m] - i * NSH, e1_src[m]))
    meta1_l1 = Meta(e1_by_core, nsub1, NSUP, N)
    nsuper1 = nsub1 // NSUP

    def flat_slab(r_glob):
        ci = np.minimum(r_glob // NSH, NC - 1)
        rl = r_glob - ci * NSH
        w = rl >> 7
        return ((w // NSUP) * (NC * 128 * NSUP)
                + (ci * 128 + (rl & 127)) * NSUP + (w % NSUP))

    e1_by_core_pm = [(d, flat_slab(s)) for d, s in e1_by_core]
    meta1_lx = Meta(e1_by_core_pm, nsub1, NSUP, rows_x)

    # ---------------- e2: consumer-sharded reviews ------------------------
    e2cnt = np.bincount(e2_dst, minlength=R)
    core_of3 = np.minimum(e3_dst // MSH, NC - 1)
    e2_srcflat = flat_slab(e2_src)
    e2_chunk = e2_srcflat // CH

    o2 = np.lexsort((e2_chunk, e2_dst))
    e2d_s, e2c_s = e2_dst[o2], e2_chunk[o2]
    rstart = np.searchsorted(e2d_s, np.arange(R + 1))
    cmin = np.full(R, 99, np.int64)
    cmax = np.full(R, 99, np.int64)
    hasr = rstart[1:] > rstart[:-1]
    if len(e2c_s):
        cmin[hasr] = e2c_s[rstart[:-1][hasr]]
        cmax[hasr] = e2c_s[rstart[1:][hasr] - 1]

    cons_lists, e2_data, e3_data, inv2_list = [], [], [], []
    for i in range(NC):
        m3 = core_of3 == i
        src3 = e3_src[m3]
        dst3 = e3_dst[m3] - i * MSH
        cons = np.unique(src3)
        key = cmin[cons].astype(np.int64) * 100 + cmax[cons]
        cons = cons[np.argsort(key, kind="stable")]
        lid = np.full(R, -1, np.int64)
        lid[cons] = np.arange(len(cons))
        cons_lists.append(cons)
        sel = lid[e2_dst] >= 0
        e2_data.append((lid[e2_dst[sel]], e2_srcflat[sel]))
        e3_data.append((dst3, lid[src3]))
        inv2_list.append((1.0 / np.maximum(e2cnt[cons], 1)).astype(np.float32))

    revcap = max(len(c) for c in cons_lists)
    nsub2 = int(_ru(-(-revcap // 128), NSUP))
    rows_rev = 128 * nsub2
    meta2 = Meta(e2_data, nsub2, NSUP, rows_x)

    def map_rev(r):
        return (r % 128) * nsub2 + (r // 128)

    e3_data_pm = [(d, map_rev(s)) for d, s in e3_data]
    meta3 = Meta(e3_data_pm, nsub3, NSUP3, rows_rev)

    # ---------------- per-core input arrays -------------------------------
    iota = np.zeros((128, 2048), np.float16)
    iota[:] = np.arange(2048).astype(np.float16)[None, :]

    in_maps = []
    for i in range(NC):
        d1, s1 = e1_by_core[i]
        idxA, dlA = meta1_l1.pack(i, d1, s1)
        d1x, s1x = e1_by_core_pm[i]
        idxB, dlB = meta1_lx.pack(i, d1x, s1x)
        inv1 = _invcnt_pmajor(d1, nsub1)
        d2, s2 = e2_data[i]
        idx2, dl2 = meta2.pack(i, d2, s2)
        inv2 = np.zeros((128, nsub2), np.float32)
        li = np.arange(len(cons_lists[i]))
        inv2[li % 128, li // 128] = inv2_list[i]
        d3, s3 = e3_data_pm[i]
        idx3, dl3 = meta3.pack(i, d3, s3)
        embl = np.zeros((nsuper1, 128, NSUP, 128), BF16)
        loc = x0b[i * NSH:(i + 1) * NSH]
        r = np.arange(NSH)
        w = r >> 7
        embl[w // NSUP, r % 128, w % NSUP, 0:64] = loc
        embl[w // NSUP, r % 128, w % NSUP, 64:128] = loc
        in_maps.append({
            "emb": emb_dup,
            "emb_local": embl,
            "idx_l1": idxA, "dl_l1": dlA,
            "idx_lx": idxB, "dl_lx": dlB,
            "inv1": inv1,
            "idx_e2": idx2, "dl_e2": dl2, "inv2": inv2,
            "idx_e3": idx3, "dl_e3": dl3,
            "iota": iota,
            "vrep": np.tile(v, (128, 1)).astype(np.float32),
            "crep": np.full((128, 1), c_sc, np.float32),
        })

    # ---------------- build device program --------------------------------
    nc = bacc.Bacc("TRN2", target_bir_lowering=False, debug=False,
                   num_devices=NC, num_swdge_queues=CFG["NQ"])

    def din(name, arr):
        return nc.dram_tensor(name, list(arr.shape),
                              mybir.dt.from_np(arr.dtype), kind="ExternalInput")

    t = {k: din(k, in_maps[0][k]) for k in in_maps[0]}
    out_t = nc.dram_tensor("out", [128, nsub3, 64], f32, kind="ExternalOutput")
    dbg_t = {}
    if CFG.get("DEBUG"):
        for nm, sh in (("d_rev", [128, nsub2, 128]),):
            dbg_t[nm] = nc.dram_tensor(nm, sh, bf16, kind="ExternalOutput")

    qstate = [0]
    with tile.TileContext(nc) as tc:
        qsems = [nc.alloc_semaphore(f"gsem{q}") for q in range(CFG["NQ"])]
        with (
            tc.tile_pool(name="psum", bufs=6, space="PSUM") as psum_p,
            tc.tile_pool(name="gather", bufs=8) as gather_p,
            tc.tile_pool(name="idx", bufs=8) as idx_p,
            tc.tile_pool(name="dloc", bufs=8) as dloc_p,
            tc.tile_pool(name="oh", bufs=4) as oh_p,
            tc.tile_pool(name="oh3", bufs=2) as oh3_p,
            tc.tile_pool(name="stage", bufs=3) as stage_p,
            tc.tile_pool(name="stage3", bufs=2) as stage3_p,
            tc.tile_pool(name="ic", bufs=3) as ic_p,
            tc.tile_pool(name="tmp", bufs=3) as tmp_p,
            tc.tile_pool(name="rhs", bufs=3) as rhs_p,
            tc.tile_pool(name="ze", bufs=3) as ze_p,
            tc.tile_pool(name="den", bufs=4) as den_p,
            tc.tile_pool(name="const", bufs=1) as const_p,
            tc.tile_pool(name="ro", bufs=3) as ro_p,
            tc.tile_pool(name="dram", bufs=1, space="DRAM") as dram_p,
        ):
            pools = {"psum": psum_p, "gather": gather_p, "idx": idx_p,
                     "dloc": dloc_p, "oh": oh_p, "oh3": oh3_p,
                     "stage": stage_p, "stage3": stage3_p, "ic": ic_p,
                     "tmp": tmp_p, "rhs": rhs_p, "ze": ze_p, "den": den_p}
            iota_t = const_p.tile([128, 2048], fp16, tag="iota")
            nc.sync.dma_start(out=iota_t[:], in_=t["iota"][:])
            vrep_t = const_p.tile([128, 64], f32, tag="vrep")
            nc.sync.dma_start(out=vrep_t[:], in_=t["vrep"][:])
            crep_t = const_p.tile([128, 1], f32, tag="crep")
            nc.sync.dma_start(out=crep_t[:], in_=t["crep"][:])

            x_loc = [dram_p.tile([nsuper1, 128, NSUP, 128], bf16, tag="x_loc",
                                 name=f"x_loc{l}") for l in range(L)]
            x_full = [dram_p.tile([nsuper1, NC * 128, NSUP, 128], bf16,
                                  tag="x_full", name=f"x_full{l}")
                      for l in range(L - 1)]
            xbar_loc = dram_p.tile([nsuper1, 128, NSUP, 128], bf16,
                                   tag="xbar_loc", name="xbar_loc")
            xbar_full = dram_p.tile([nsuper1, NC * 128, NSUP, 128], bf16,
                                    tag="xbar_full", name="xbar_full")
            rev_loc = dram_p.tile([128, nsub2, 128], bf16, tag="rev_loc",
                                  name="rev_loc")

            # ---- propagation layers ----
            for l in range(L):
                if l == 0:
                    src_view = t["emb"][:]
                    meta_l = meta1_l1
                    idx_l, dl_l = t["idx_l1"], t["dl_l1"]
                else:
                    xf = x_loc[l - 1] if NC == 1 else x_full[l - 1]
                    src_view = xf[:].rearrange("s a w d -> (s a w) d")
                    meta_l = meta1_lx
                    idx_l, dl_l = t["idx_lx"], t["dl_lx"]
                on_sup = None
                if l < L - 1 and NC > 1:
                    xl, xfl = x_loc[l], x_full[l]

                    def on_sup(s, xl=xl, xfl=xfl):
                        nc.gpsimd.collective_compute(
                            "AllGather", mybir.AluOpType.bypass,
                            replica_groups=[list(range(NC))],
                            ins=[xl[s]], outs=[xfl[s]])
                xl = x_loc[l]
                _emit_phase(nc, pools, meta_l, src_view, idx_l[:], dl_l[:],
                            None, invcnt_t=t["inv1"][:], iota_t=iota_t,
                            qstate=qstate, qsems=qsems, on_super=on_sup,
                            out_ap_fn=lambda s, xl=xl: xl[s])

            # ---- readout mean (per super slab) ----
            for s in range(nsuper1):
                for half in range(2):
                    w0, wn = half * 8, 8
                    t0 = ro_p.tile([128, 8, 128], bf16, tag="roemb")
                    nc.sync.dma_start(out=t0[:],
                                      in_=t["emb_local"][s, :, w0:w0 + wn, :])
                    acc = ro_p.tile([128, 8, 128], f32, tag="roacc")
                    tl = ro_p.tile([128, 8, 128], bf16, tag="rold0")
                    nc.sync.dma_start(out=tl[:],
                                      in_=x_loc[0][s, :, w0:w0 + wn, :])
                    nc.vector.tensor_tensor(out=acc[:], in0=t0[:], in1=tl[:],
                                            op=mybir.AluOpType.add)
                    for l in range(1, L):
                        tl = ro_p.tile([128, 8, 128], bf16, tag="rold")
                        nc.sync.dma_start(out=tl[:],
                                          in_=x_loc[l][s, :, w0:w0 + wn, :])
                        nc.vector.tensor_tensor(out=acc[:], in0=acc[:],
                                                in1=tl[:],
                                                op=mybir.AluOpType.add)
                    accb = ro_p.tile([128, 8, 128], bf16, tag="roaccb")
                    nc.vector.tensor_scalar(out=accb[:], in0=acc[:],
                                            scalar1=1.0 / (L + 1),
                                            scalar2=None,
                                            op0=mybir.AluOpType.mult)
                    nc.sync.dma_start(out=xbar_loc[s, :, w0:w0 + wn, :],
                                      in_=accb[:])
                if NC > 1:
                    nc.gpsimd.collective_compute(
                        "AllGather", mybir.AluOpType.bypass,
                        replica_groups=[list(range(NC))],
                        ins=[xbar_loc[s]], outs=[xbar_full[s]])

            # ---- e2: review representations (adds ones column) ----
            _emit_phase(nc, pools, meta2,
                        (xbar_loc if NC == 1 else xbar_full)[:]
                        .rearrange("s a w d -> (s a w) d"),
                        t["idx_e2"][:], t["dl_e2"][:], rev_loc,
                        invcnt_t=t["inv2"][:], iota_t=iota_t,
                        qstate=qstate, qsems=qsems, ones_col=True)

            if CFG.get("DEBUG"):
                for nm, srcv in (("d_rev", rev_loc[:]),):
                    nsb = srcv.shape[1]
                    for w0 in range(0, nsb, 16):
                        wn = min(16, nsb - w0)
                        bt = stage_p.tile([128, 16, 128], bf16, tag="dbgb",
                                          name=f"dbgb_{nm}_{w0}")
                        nc.sync.dma_start(out=bt[:, :wn, :],
                                          in_=srcv[:, w0:w0 + wn, :])
                        nc.sync.dma_start(out=dbg_t[nm][:, w0:w0 + wn, :],
                                          in_=bt[:, :wn, :])

            # ---- e3: edge-softmax attention ----
            _emit_phase(nc, pools, meta3,
                        rev_loc[:].rearrange("p w d -> (p w) d"),
                        t["idx_e3"][:], t["dl_e3"][:], out_t,
                        invcnt_t=None, iota_t=iota_t,
                        e3=(vrep_t, crep_t), qstate=qstate, qsems=qsems)

    nc.compile()
    if not RUN:
        meta_out = dict(nsub3=nsub3, MSH=MSH)
        return nc, in_maps, meta_out

    res = run_bass_kernel_spmd(
        nc, in_maps, core_ids=list(range(NC)),
        trace=CFG["TRACE"] or os.environ.get("GNN_TRACE") == "1")
    _LAST["exec_ns"] = res.exec_time_ns
    _LAST["profile_json"] = res.profile_json
    _LAST["results"] = res.results

    out = np.empty((M, D), np.float32)
    for i in range(NC):
        o = res.results[i]["out"]
        r = np.arange(MSH)
        out[i * MSH:(i + 1) * MSH] = o[r % 128, r // 128]
    return out
